# revision 21
# baseline (speedup 1.0000x reference)
"""Trainium2 Bass kernel for nn_Attention_org_29961691856973.

Sharding: pure data-parallel over batch (B=8 -> 8 NeuronCores, 1 sample each).
All weights replicated; no collectives.

Per-core pipeline (channel-major layout: channels on partitions, hw=1024 free):
  stage 1: A_t = W_t @ X (f32r matmuls), depthwise 3x3 via fused MAC chains on
           DVE/GpSimd (flat layout + zero guards + seam fixups), l2norm rows,
           PE transposes Q,K -> n-major, S^T = K^T' Q^T, instance-norm stats via
           ACT accum + ones-matmul partition reduction, exp fused w/ scale/bias,
           ctx = (E^T)^T [V | 1] with softmax row-sums from the ones column.
  stage 2 (x4 branches): grouped conv folded into duplicated 1x1 weights
           (host-prepped), same dwconv machinery, attn^T = ctxT' qT, instnorm +
           softmax via the same transposed-E trick, o = (E2^T)^T ctx, proj.
"""

import math
import os

import numpy as np

P = 128
HW = 1024
KV = 960
KVP = 1024
NKT = KVP // P  # 8
CH = [64, 128, 256, 512]
CTS = [1, 1, 2, 4]  # channel tiles per branch
SCALE = 1.0 / math.sqrt(KV)
EPS = 1e-5
APAD_W = 1128  # 34 guard + 1024 + 34 guard + slack (max AP window start 98+1024)
GL = 34  # left guard size / interior start

# 960 = 7*128 + 64 tiling helper
def dtiles():
    return [(t * P, P if t < 7 else 64) for t in range(8)]


_CACHE = {}


def _build_nc():
    import concourse.bass as bass
    import concourse.mybir as mybir
    import concourse.tile as tile
    from concourse import bacc
    from concourse.masks import make_identity
    from contextlib import ExitStack
    from itertools import cycle

    f32 = mybir.dt.float32
    bf16 = mybir.dt.bfloat16
    AF = mybir.ActivationFunctionType
    OP = mybir.AluOpType

    nc = bacc.Bacc(None, target_bir_lowering=False)

    def r(ap):
        return ap

    # ------------- DRAM I/O -------------
    x_all = nc.dram_tensor("x_all", [KVP, HW], bf16, kind="ExternalInput")
    wT = {
        t: nc.dram_tensor(f"wT_{t}", [KVP, KVP], bf16, kind="ExternalInput")
        for t in "qkv"
    }
    taps1d = {
        t: nc.dram_tensor(f"taps_{t}", [NKT, P, 15], f32, kind="ExternalInput")
        for t in "qkv"
    }
    xb_d, waT_d, wbT_d, tpa_d, tpb_d, wpT_d, out_d = [], [], [], [], [], [], []
    for i, c in enumerate(CH):
        ct = CTS[i]
        pb = min(P, c)
        xb_d.append(nc.dram_tensor(f"x{i + 1}", [c, HW], bf16, kind="ExternalInput"))
        waT_d.append(nc.dram_tensor(f"waT{i + 1}", [c, c], bf16, kind="ExternalInput"))
        wbT_d.append(nc.dram_tensor(f"wbT{i + 1}", [c, c], bf16, kind="ExternalInput"))
        tpa_d.append(
            nc.dram_tensor(f"tapsa{i + 1}", [ct, pb, 15], f32, kind="ExternalInput")
        )
        tpb_d.append(
            nc.dram_tensor(f"tapsb{i + 1}", [ct, pb, 15], f32, kind="ExternalInput")
        )
        wpT_d.append(nc.dram_tensor(f"wpT{i + 1}", [c, c], bf16, kind="ExternalInput"))
        out_d.append(nc.dram_tensor(f"o{i + 1}", [c, HW], f32, kind="ExternalOutput"))

    dwcyc = cycle(["v", "v", "g"])
    open_cms = {}

    with tile.TileContext(nc) as tc, ExitStack() as top:

        def popen(name, bufs=1, space="SBUF", side=None):
            cm = tc.tile_pool(name=name, bufs=bufs, space=space, side=side)
            open_cms[name] = cm
            return cm.__enter__()

        def pclose(*names):
            for n in names:
                open_cms.pop(n).__exit__(None, None, None)

        const = top.enter_context(tc.tile_pool(name="const", bufs=1))
        scrp = top.enter_context(tc.tile_pool(name="scr", bufs=2))
        smallp = top.enter_context(tc.tile_pool(name="small", bufs=4))
        statp = top.enter_context(tc.tile_pool(name="statp", bufs=1))
        p_mm = top.enter_context(tc.tile_pool(name="p_mm", bufs=2, space="PSUM"))
        p_sm = top.enter_context(tc.tile_pool(name="p_sm", bufs=2, space="PSUM"))

        ident = const.tile([P, P], f32)
        make_identity(nc, ident)
        ones = const.tile([P, P], f32)
        nc.vector.memset(ones, 1.0)
        zc = const.tile([P, 1], f32)
        nc.vector.memset(zc, 0.0)
        ec = const.tile([P, 1], f32)
        nc.vector.memset(ec, EPS)
        ident_bf = const.tile([P, P], bf16)
        make_identity(nc, ident_bf)
        ones_bf = const.tile([P, 8], bf16)
        nc.vector.memset(ones_bf, 1.0)

        # ---------- helpers ----------
        def dwconv_chain(qdst, pairs, pt, ekey):
            """qdst: [pt, 1024] output. pairs: [(apad, taps[pt,15])].

            DVE: fused scalar_tensor_tensor MACs (1 op/tap).
            GpSimd: Pool lacks the STT form -> tensor_scalar_mul + tensor_tensor.
            """
            e = nc.vector if ekey == "v" else nc.gpsimd
            qv = qdst.rearrange("p (y x) -> p y x", x=32)
            tmp = None
            if ekey != "v":
                tmp = scrp.tile([P, HW], f32, tag="gtmp")

            def mac(dst, src, w):
                if ekey == "v":
                    e.scalar_tensor_tensor(dst, src, w, dst, OP.mult, OP.add)
                else:
                    tv = tmp[:pt, : src.shape[-1]] if len(src.shape) == 2 else (
                        tmp[:pt, 0 : 32 * src.shape[1]].rearrange(
                            "p (y x) -> p y x", x=src.shape[2]))
                    e.tensor_scalar_mul(tv, src, w)
                    e.tensor_tensor(dst, dst, tv, OP.add)

            first = True
            for apad, tp in pairs:
                for dy in (-1, 0, 1):
                    for dx in (-1, 0, 1):
                        off = 32 * dy + dx
                        src = apad[:pt, GL + off : GL + off + HW]
                        w = tp[:pt, (dy + 1) * 3 + (dx + 1) : (dy + 1) * 3 + (dx + 1) + 1]
                        if first:
                            e.tensor_scalar_mul(qdst, src, w)
                            first = False
                        else:
                            mac(qdst, src, w)
            # seam fixups (x-wraparound corrections at columns 0 and 31)
            for apad, tp in pairs:
                for dyi, dy in enumerate((-1, 0, 1)):
                    wL = apad[:pt, 32 * dy + 33 : 32 * dy + 33 + HW].rearrange(
                        "p (y x) -> p y x", x=32
                    )
                    mac(qv[:, :, 0], wL[:, :, 0], tp[:pt, 9 + dyi : 10 + dyi])
                    wR = apad[:pt, 32 * dy + 66 : 32 * dy + 66 + HW].rearrange(
                        "p (y x) -> p y x", x=32
                    )
                    mac(qv[:, :, 31], wR[:, :, 0], tp[:pt, 12 + dyi : 13 + dyi])

        def evict_to_apad(apad, ps, pt, ekey):
            e = nc.vector if ekey == "v" else nc.gpsimd
            e.memset(apad[:pt, 0:GL], 0.0)
            e.memset(apad[:pt, GL + HW : GL + HW + GL], 0.0)
            nc.scalar.copy(apad[:pt, GL : GL + 512], ps[:pt, 0:512])
            nc.scalar.copy(apad[:pt, GL + 512 : GL + HW], ps[:pt, 512:1024])

        def l2norm_rows(qslice, pt):
            """qslice [pt, 1024] -> divide rows by max(||row||, 1e-12)."""
            scr = scrp.tile([P, HW], f32, tag="scr")
            ss = smallp.tile([P, 1], f32, tag="ss")
            nc.scalar.activation(
                scr[:pt], qslice, AF.Square, bias=zc[:pt, 0:1],
                accum_out=ss[:pt, 0:1]
            )
            nc.scalar.activation(ss[:pt, 0:1], ss[:pt, 0:1], AF.Sqrt,
                                 bias=zc[:pt, 0:1])
            nc.vector.tensor_scalar_max(ss[:pt, 0:1], ss[:pt, 0:1], 1e-12)
            rn = smallp.tile([P, 1], f32, tag="rn")
            nc.vector.reciprocal(rn[:pt, 0:1], ss[:pt, 0:1])
            nc.scalar.activation(qslice, qslice, AF.Copy, scale=rn[:pt, 0:1])

        def instnorm_scalars(tiles, nvalid, name):
            """tiles: list of (ap [pt, w], pt). Returns (escale, ebias) [128,1]."""
            stats_ps = p_sm.tile([P, 8], f32, tag="d")
            for t, (ap, pt) in enumerate(tiles):
                stp = smallp.tile([P, 2], f32, tag="stp")
                scr = scrp.tile([P, HW], f32, tag="scr")
                w = ap.shape[-1]
                nc.scalar.activation(
                    scr[:pt, :w], ap, AF.Square, bias=zc[:pt, 0:1],
                    accum_out=stp[:pt, 1:2]
                )
                nc.scalar.activation(
                    scr[:pt, :w], ap, AF.Identity, bias=zc[:pt, 0:1],
                    accum_out=stp[:pt, 0:1]
                )
                nc.tensor.matmul(
                    stats_ps[:, 0:2], ones[:pt, :], stp[:pt, 0:2],
                    start=(t == 0), stop=(t == len(tiles) - 1),
                )
            st = statp.tile([P, 8], f32, name=f"st_{name}")
            nc.vector.tensor_copy(st[:, 0:2], stats_ps[:, 0:2])
            m_s = st[:, 2:3]
            es2 = st[:, 3:4]
            var = st[:, 4:5]
            nc.vector.tensor_scalar_mul(m_s, st[:, 0:1], SCALE / nvalid)
            nc.vector.tensor_scalar_mul(es2, st[:, 1:2], SCALE * SCALE / nvalid)
            nc.vector.tensor_tensor(var, m_s, m_s, OP.mult)
            nc.vector.tensor_tensor(var, es2, var, OP.subtract)
            sd = st[:, 5:6]
            nc.scalar.activation(sd, var, AF.Sqrt, bias=ec[:, 0:1])
            rstd = st[:, 6:7]
            nc.vector.reciprocal(rstd, sd)
            escale = statp.tile([P, 1], f32, name=f"esc_{name}")
            ebias = statp.tile([P, 1], f32, name=f"ebi_{name}")
            nc.vector.tensor_scalar_mul(escale, rstd, SCALE)
            nc.vector.tensor_tensor(ebias, m_s, rstd, OP.mult)
            nc.vector.tensor_scalar_mul(ebias, ebias, -1.0)
            return escale, ebias

        # ================= stage 1 =================
        pv = popen("pv", side="right")  # v: until ctx done
        pqk = popen("pqk")  # q,k: until transposes done
        q_sb = pqk.tile([P, NKT, HW], f32, name="q_sb")
        k_sb = pqk.tile([P, NKT, HW], f32, name="k_sb")
        v_bf = pv.tile([P, NKT, HW + 8], bf16, name="v_bf")

        pA = popen("pA")
        pw = popen("pw", bufs=2)
        pap = popen("pap", bufs=4)
        x_sb = pA.tile([P, NKT, HW], bf16, name="x_sb")
        nc.sync.dma_start(x_sb[:], x_all.rearrange("(kt p) n -> p kt n", p=P))
        taps1 = {}
        for t in "qkv":
            tt = pA.tile([P, NKT, 15], f32, name=f"taps1{t}")
            nc.sync.dma_start(tt[:], taps1d[t].rearrange("kt p f -> p kt f"))
            taps1[t] = tt

        for t in "qkv":
            wm = pw.tile([P, NKT, KVP], bf16, tag="wm")
            nc.sync.dma_start(wm[:], wT[t].rearrange("(ko p) m -> p ko m", p=P))
            for m in range(NKT):
                ps = p_mm.tile([P, 1024], f32, tag="mm")
                for kt in range(NKT):
                    nc.tensor.matmul(
                        ps[:, 0:512], r(wm[:, kt, m * P : (m + 1) * P]),
                        r(x_sb[:, kt, 0:512]),
                        start=(kt == 0), stop=(kt == NKT - 1),
                    )
                    nc.tensor.matmul(
                        ps[:, 512:1024], r(wm[:, kt, m * P : (m + 1) * P]),
                        r(x_sb[:, kt, 512:1024]),
                        start=(kt == 0), stop=(kt == NKT - 1),
                    )
                ek = next(dwcyc)
                apad = pap.tile([P, APAD_W], f32, tag="apad")
                evict_to_apad(apad, ps, P, ek)
                if t == "v":
                    vtmp = pw.tile([P, HW], f32, tag="vtmp")
                    dwconv_chain(vtmp[:, 0:HW], [(apad, taps1[t][:, m, :])], P, ek)
                    nc.scalar.copy(v_bf[:, m, 0:HW], vtmp[:, 0:HW])
                    nc.vector.memset(v_bf[:, m, HW : HW + 1], 1.0)
                else:
                    dst = q_sb if t == "q" else k_sb
                    dwconv_chain(dst[:, m, 0:HW], [(apad, taps1[t][:, m, :])], P, ek)
                    l2norm_rows(dst[:, m, 0:HW], P)
        pclose("pap", "pw", "pA")

        # ---- transposes Q,K -> n-major ----
        pT = popen("pT", side="right")
        qT = pT.tile([P, NKT, KVP], bf16, name="qT")
        kT = pT.tile([P, NKT, KVP], bf16, name="kT")
        for src, dstT in ((q_sb, qT), (k_sb, kT)):
            for j in range(NKT):
                pst = p_mm.tile([P, 1024], f32, tag="mm")
                for m in range(NKT):
                    nc.tensor.transpose(
                        pst[:, m * P : (m + 1) * P],
                        src[:, m, j * P : (j + 1) * P],
                        ident,
                    )
                nc.scalar.copy(dstT[:, j, 0:512], pst[:, 0:512])
                nc.scalar.copy(dstT[:, j, 512:1024], pst[:, 512:1024])
        pclose("pqk")

        # ---- S^T = (K^T)' @ Q^T ; tiles over d (960) ----
        pctx = popen("pctx")  # opened early for LIFO: outlives psT
        ctx = pctx.tile([P, 8, HW], bf16, name="ctx")
        peT = popen("peT")
        eT = peT.tile([P, 8, KV], bf16, name="eT")
        psT = popen("psT")
        sT = psT.tile([P, 8, KV], f32, name="sT")
        for t, (ds, pt) in enumerate(dtiles()):
            ps = p_mm.tile([P, 1024], f32, tag="mm")
            for j in range(NKT):
                lh = kT[:, j, ds : ds + pt]
                nc.tensor.matmul(
                    ps[:pt, 0:512], r(lh), r(qT[:, j, 0:512]),
                    start=(j == 0), stop=(j == NKT - 1),
                )
                nc.tensor.matmul(
                    ps[:pt, 512:KV], r(lh), r(qT[:, j, 512:KV]),
                    start=(j == 0), stop=(j == NKT - 1),
                )
            nc.scalar.copy(sT[:pt, t, 0:KV], ps[:pt, 0:KV])
        pclose("pT")

        esc1, ebi1 = instnorm_scalars(
            [(sT[:pt, t, 0:KV], pt) for t, (ds, pt) in enumerate(dtiles())],
            KV * KV, "s1",
        )
        for t, (ds, pt) in enumerate(dtiles()):
            nc.scalar.activation(
                eT[:pt, t, 0:KV], sT[:pt, t, 0:KV], AF.Exp,
                bias=ebi1[:pt, 0:1], scale=esc1[:pt, 0:1],
            )
        pclose("psT")

        # ---- ctx = (E^T)' @ [V | 1] with row-sum normalization ----
        for m, (ms, mw) in enumerate(dtiles()):
            ps = p_mm.tile([P, 1024], f32, tag="mm")
            psd = p_sm.tile([P, 8], f32, tag="d")
            for t, (ds, pt) in enumerate(dtiles()):
                lh = eT[:pt, t, ms : ms + mw]
                st_, sp_ = (t == 0), (t == 7)
                nc.tensor.matmul(ps[:mw, 0:512], lh, v_bf[:pt, t, 0:512],
                                 start=st_, stop=sp_)
                nc.tensor.matmul(ps[:mw, 512:1024], lh, v_bf[:pt, t, 512:1024],
                                 start=st_, stop=sp_)
                nc.tensor.matmul(psd[:mw, 0:1], lh, v_bf[:pt, t, HW : HW + 1],
                                 start=st_, stop=sp_)
            rd = smallp.tile([P, 1], f32, tag="rd")
            nc.vector.reciprocal(rd[:mw, 0:1], psd[:mw, 0:1])
            nc.scalar.activation(ctx[:mw, m, 0:512], ps[:mw, 0:512], AF.Copy,
                                 scale=rd[:mw, 0:1])
            nc.scalar.activation(ctx[:mw, m, 512:1024], ps[:mw, 512:1024], AF.Copy,
                                 scale=rd[:mw, 0:1])
        pclose("peT")
        pclose("pv")

        # ---- ctxT ----
        pctxT = popen("pctxT")
        ctxT = pctxT.tile([P, NKT, KV], bf16, name="ctxT")
        for j in range(NKT):
            pst = p_mm.tile([P, 1024], bf16, tag="mm")
            for m, (ms, mw) in enumerate(dtiles()):
                nc.tensor.transpose(
                    pst[:, ms : ms + mw], ctx[:mw, m, j * P : (j + 1) * P],
                    ident_bf[:mw, :mw],
                )
            nc.scalar.copy(ctxT[:, j, 0:KV], pst[:, 0:KV])

        # ================= branches: phase A =================
        pqTb = popen("pqTb")
        qTb = []
        pbA = popen("pbA", bufs=2)
        pbw = popen("pbw", bufs=2)
        pap2 = popen("pap2", bufs=4)
        for i, c in enumerate(CH):
            ct, pb = CTS[i], min(P, c)
            xb = pbA.tile([P, 4, HW], bf16, tag="xb")
            nc.sync.dma_start(
                xb[:pb, :ct, :], xb_d[i].rearrange("(ct p) n -> p ct n", p=pb)
            )
            wa = pbw.tile([P, 4, 512], bf16, tag="wab")
            nc.sync.dma_start(
                wa[:pb, :ct, :c], waT_d[i].rearrange("(kt p) m -> p kt m", p=pb)
            )
            wb = pbw.tile([P, 4, 512], bf16, tag="wab")
            nc.sync.dma_start(
                wb[:pb, :ct, :c], wbT_d[i].rearrange("(kt p) m -> p kt m", p=pb)
            )
            tpa = pbA.tile([P, 4, 15], f32, tag="tp")
            nc.sync.dma_start(tpa[:pb, :ct, :], tpa_d[i].rearrange("ct p f -> p ct f"))
            tpb = pbA.tile([P, 4, 15], f32, tag="tp")
            nc.sync.dma_start(tpb[:pb, :ct, :], tpb_d[i].rearrange("ct p f -> p ct f"))

            qb = pbA.tile([P, 4, HW], f32, tag="qb")
            for m in range(ct):
                mw = pb if ct == 1 else P
                pads = []
                for wsb in (wa, wb):
                    ps = p_mm.tile([P, 1024], f32, tag="mm")
                    for kt in range(ct):
                        nc.tensor.matmul(
                            ps[:mw, 0:512],
                            r(wsb[:pb, kt, m * P : m * P + mw]),
                            r(xb[:pb, kt, 0:512]),
                            start=(kt == 0), stop=(kt == ct - 1),
                        )
                        nc.tensor.matmul(
                            ps[:mw, 512:1024],
                            r(wsb[:pb, kt, m * P : m * P + mw]),
                            r(xb[:pb, kt, 512:1024]),
                            start=(kt == 0), stop=(kt == ct - 1),
                        )
                    ek = next(dwcyc)
                    apad = pap2.tile([P, APAD_W], f32, tag="apad")
                    evict_to_apad(apad, ps, mw, ek)
                    pads.append((apad, ek))
                ek = pads[0][1]
                dwconv_chain(
                    qb[:mw, m, 0:HW],
                    [(pads[0][0], tpa[:pb, m, :]), (pads[1][0], tpb[:pb, m, :])],
                    mw, ek,
                )
                l2norm_rows(qb[:mw, m, 0:HW], mw)

            qt = pqTb.tile([P, NKT, c], bf16, name=f"qTb{i}")
            for j in range(NKT):
                pst = p_mm.tile([P, 1024], f32, tag="mm")
                for m in range(ct):
                    mw = pb if ct == 1 else P
                    nc.tensor.transpose(
                        pst[:, m * P : m * P + mw],
                        qb[:mw, m, j * P : (j + 1) * P],
                        ident[:mw, :mw],
                    )
                nc.scalar.copy(qt[:, j, 0:c], pst[:, 0:c])
            qTb.append(qt)
        pclose("pap2", "pbw", "pbA")

        # ================= branches: phase B1 (attn^T + stats) ============
        ps2T = popen("ps2T", side="right")
        s2T, escb, ebib = [], [], []
        for i, c in enumerate(CH):
            st2 = ps2T.tile([P, 8, c], bf16, name=f"s2T{i}")
            for t, (ds, pt) in enumerate(dtiles()):
                ps = p_mm.tile([P, 1024], f32, tag="mm")
                for j in range(NKT):
                    nc.tensor.matmul(
                        ps[:pt, 0:c], r(ctxT[:, j, ds : ds + pt]),
                        r(qTb[i][:, j, 0:c]),
                        start=(j == 0), stop=(j == NKT - 1),
                    )
                nc.scalar.copy(st2[:pt, t, 0:c], ps[:pt, 0:c])
            s2T.append(st2)
            es, eb = instnorm_scalars(
                [(st2[:pt, t, 0:c], pt) for t, (ds, pt) in enumerate(dtiles())],
                KV * c, f"b{i}",
            )
            escb.append(es)
            ebib.append(eb)
        pclose("pqTb", "pctxT")

        # ================= branches: phase B2 (exp, o, proj, out) =========
        for i, c in enumerate(CH):
            ct, pb = CTS[i], min(P, c)
            st2 = s2T[i]
            for t, (ds, pt) in enumerate(dtiles()):
                nc.scalar.activation(
                    st2[:pt, t, 0:c], st2[:pt, t, 0:c], AF.Exp,
                    bias=ebib[i][:pt, 0:1], scale=escb[i][:pt, 0:1],
                )
            with tc.tile_pool(name=f"pb2_{i}", bufs=1) as pb2:
                ob = pb2.tile([pb, ct, HW], bf16, name=f"ob{i}")
                for m in range(ct):
                    mw = pb if ct == 1 else P
                    ps = p_mm.tile([P, 1024], f32, tag="mm")
                    psd = p_sm.tile([P, 8], f32, tag="d")
                    for t, (ds, pt) in enumerate(dtiles()):
                        lh = st2[:pt, t, m * P : m * P + mw]
                        st_, sp_ = (t == 0), (t == 7)
                        nc.tensor.matmul(ps[:mw, 0:512], r(lh),
                                         r(ctx[:pt, t, 0:512]),
                                         start=st_, stop=sp_)
                        nc.tensor.matmul(ps[:mw, 512:1024], r(lh),
                                         r(ctx[:pt, t, 512:1024]),
                                         start=st_, stop=sp_)
                        nc.tensor.matmul(psd[:mw, 0:1], lh, ones_bf[:pt, 0:1],
                                         start=st_, stop=sp_)
                    rd = smallp.tile([P, 1], f32, tag="rd")
                    nc.vector.reciprocal(rd[:mw, 0:1], psd[:mw, 0:1])
                    nc.scalar.activation(ob[:mw, m, 0:512], ps[:mw, 0:512],
                                         AF.Copy, scale=rd[:mw, 0:1])
                    nc.scalar.activation(ob[:mw, m, 512:1024], ps[:mw, 512:1024],
                                         AF.Copy, scale=rd[:mw, 0:1])
                # proj
                wp = pb2.tile([pb, ct, c], bf16, name=f"wp{i}")
                nc.sync.dma_start(
                    wp[:], wpT_d[i].rearrange("(kt p) m -> p kt m", p=pb)
                )
                outb = pb2.tile([pb, ct, HW], f32, name=f"outb{i}")
                for m in range(ct):
                    mw = pb if ct == 1 else P
                    ps = p_mm.tile([P, 1024], f32, tag="mm")
                    for kt in range(ct):
                        nc.tensor.matmul(
                            ps[:mw, 0:512], r(wp[:pb, kt, m * P : m * P + mw]),
                            r(ob[:pb, kt, 0:512]),
                            start=(kt == 0), stop=(kt == ct - 1),
                        )
                        nc.tensor.matmul(
                            ps[:mw, 512:1024], r(wp[:pb, kt, m * P : m * P + mw]),
                            r(ob[:pb, kt, 512:1024]),
                            start=(kt == 0), stop=(kt == ct - 1),
                        )
                    nc.scalar.copy(outb[:mw, m, 0:512], ps[:mw, 0:512])
                    nc.scalar.copy(outb[:mw, m, 512:1024], ps[:mw, 512:1024])
                nc.sync.dma_start(
                    out_d[i].rearrange("(ct p) n -> p ct n", p=pb), outb[:]
                )
        pclose("ps2T", "pctx")

    nc.compile()
    return nc


def _prep_taps(w, c):
    """w: [c, 3, 3] -> [c, 15]: 9 taps + 3 negated dx=-1 + 3 negated dx=+1."""
    t = np.zeros((c, 15), np.float32)
    t[:, 0:9] = w.reshape(c, 9)
    for dyi in range(3):
        t[:, 9 + dyi] = -w[:, dyi, 0]
        t[:, 12 + dyi] = -w[:, dyi, 2]
    return t


def _host_prep(inputs):
    """Build the shared (weight) tensors + per-core input maps."""
    import ml_dtypes

    f = np.float32
    bf = ml_dtypes.bfloat16
    shared = {}
    for t, wk, dk in (("q", "w_mheadq", "w_qc"), ("k", "w_mheadk", "w_kc"),
                      ("v", "w_mheadv", "w_vc")):
        w = np.zeros((KVP, KVP), bf)
        w[:KV, :KV] = np.asarray(inputs[wk]).astype(f).T.astype(bf)
        shared[f"wT_{t}"] = w
        tp = np.zeros((KVP, 15), f)
        tp[:KV] = _prep_taps(np.asarray(inputs[dk])[:, 0].astype(f), KV)
        shared[f"taps_{t}"] = np.ascontiguousarray(tp.reshape(NKT, P, 15))
    for i, c in enumerate(CH):
        ct, pb = CTS[i], min(P, c)
        wmh = np.asarray(inputs[f"w_mhead{i + 1}"]).astype(f)
        idx = np.arange(c)
        wa = wmh[2 * (idx // 2)]
        wb = wmh[2 * (idx // 2) + 1]
        shared[f"waT{i + 1}"] = np.ascontiguousarray(wa.T).astype(bf)
        shared[f"wbT{i + 1}"] = np.ascontiguousarray(wb.T).astype(bf)
        wq = np.asarray(inputs[f"w_q{i + 1}"]).astype(f)  # [c, 2, 3, 3]
        shared[f"tapsa{i + 1}"] = np.ascontiguousarray(
            _prep_taps(wq[:, 0], c).reshape(ct, pb, 15))
        shared[f"tapsb{i + 1}"] = np.ascontiguousarray(
            _prep_taps(wq[:, 1], c).reshape(ct, pb, 15))
        shared[f"wpT{i + 1}"] = np.ascontiguousarray(
            np.asarray(inputs[f"w_proj{i + 1}"]).astype(f).T).astype(bf)

    in_maps = []
    B = np.asarray(inputs["emb_all"]).shape[0]
    for s in range(B):
        m = dict(shared)
        xa = np.zeros((KVP, HW), bf)
        xa[:KV] = np.asarray(inputs["emb_all"])[s].reshape(KV, HW).astype(bf)
        m["x_all"] = xa
        for i, c in enumerate(CH):
            m[f"x{i + 1}"] = np.ascontiguousarray(
                np.asarray(inputs[f"emb{i + 1}"])[s].reshape(c, HW).astype(f)
            ).astype(bf)
        in_maps.append(m)
    return in_maps


def kernel(**inputs):
    from concourse.bass_utils import run_bass_kernel_spmd

    if "nc" not in _CACHE:
        _CACHE["nc"] = _build_nc()
    nc = _CACHE["nc"]

    in_maps = _host_prep(inputs)
    trace = os.environ.get("KERNEL_TRACE", "0") == "1"
    kw = {}
    if trace:
        kw = dict(trace=True, trace_cores=[0])
    res = run_bass_kernel_spmd(nc, in_maps, core_ids=list(range(8)), **kw)
    if trace and res.exec_time_ns is not None:
        print(f"HW exec time: {res.exec_time_ns} ns")
        if res.instructions_and_trace is not None:
            print("trace:", res.instructions_and_trace[1])
        _CACHE["last_result"] = res

    B = len(in_maps)
    outs = []
    for i, c in enumerate(CH):
        o = np.stack([res.results[s][f"o{i + 1}"] for s in range(B)])
        outs.append(o.reshape(B, c, 32, 32).astype(np.float32))
    return tuple(outs)


# revision 22
# speedup vs baseline: 2.4456x; 2.4456x over previous
"""Trainium2 Bass kernel for nn_Attention_org_29961691856973.

Sharding: pure data-parallel over batch (B=8 -> 8 NeuronCores, 1 sample each).
All weights replicated; no collectives.

Per-core pipeline (channel-major layout: channels on partitions, hw=1024 free):
  stage 1: A_t = W_t @ X (f32r matmuls), depthwise 3x3 via fused MAC chains on
           DVE/GpSimd (flat layout + zero guards + seam fixups), l2norm rows,
           PE transposes Q,K -> n-major, S^T = K^T' Q^T, instance-norm stats via
           ACT accum + ones-matmul partition reduction, exp fused w/ scale/bias,
           ctx = (E^T)^T [V | 1] with softmax row-sums from the ones column.
  stage 2 (x4 branches): grouped conv folded into duplicated 1x1 weights
           (host-prepped), same dwconv machinery, attn^T = ctxT' qT, instnorm +
           softmax via the same transposed-E trick, o = (E2^T)^T ctx, proj.
"""

import math
import os

import numpy as np

P = 128
HW = 1024
KV = 960
KVP = 1024
NKT = KVP // P  # 8
CH = [64, 128, 256, 512]
CTS = [1, 1, 2, 4]  # channel tiles per branch
SCALE = 1.0 / math.sqrt(KV)
EPS = 1e-5
APAD_W = 1128  # 34 guard + 1024 + 34 guard + slack (max AP window start 98+1024)
GL = 34  # left guard size / interior start

# 960 = 7*128 + 64 tiling helper
def dtiles():
    return [(t * P, P if t < 7 else 64) for t in range(8)]


_CACHE = {}


def _build_nc():
    import concourse.bass as bass
    import concourse.mybir as mybir
    import concourse.tile as tile
    from concourse import bacc
    from concourse.masks import make_identity
    from contextlib import ExitStack
    from itertools import cycle

    f32 = mybir.dt.float32
    bf16 = mybir.dt.bfloat16
    AF = mybir.ActivationFunctionType
    OP = mybir.AluOpType

    nc = bacc.Bacc(None, target_bir_lowering=False)

    def r(ap):
        return ap

    # ------------- DRAM I/O -------------
    x_all = nc.dram_tensor("x_all", [KVP, HW], bf16, kind="ExternalInput")
    wT = {
        t: nc.dram_tensor(f"wT_{t}", [KVP, KVP], bf16, kind="ExternalInput")
        for t in "qkv"
    }
    taps1d = {
        t: nc.dram_tensor(f"taps_{t}", [NKT, P, 15], f32, kind="ExternalInput")
        for t in "qkv"
    }
    xb_d, waT_d, wbT_d, tpa_d, tpb_d, wpT_d, out_d = [], [], [], [], [], [], []
    for i, c in enumerate(CH):
        ct = CTS[i]
        pb = min(P, c)
        xb_d.append(nc.dram_tensor(f"x{i + 1}", [c, HW], bf16, kind="ExternalInput"))
        waT_d.append(nc.dram_tensor(f"waT{i + 1}", [c, c], bf16, kind="ExternalInput"))
        wbT_d.append(nc.dram_tensor(f"wbT{i + 1}", [c, c], bf16, kind="ExternalInput"))
        tpa_d.append(
            nc.dram_tensor(f"tapsa{i + 1}", [ct, pb, 15], f32, kind="ExternalInput")
        )
        tpb_d.append(
            nc.dram_tensor(f"tapsb{i + 1}", [ct, pb, 15], f32, kind="ExternalInput")
        )
        wpT_d.append(nc.dram_tensor(f"wpT{i + 1}", [c, c], bf16, kind="ExternalInput"))
        out_d.append(nc.dram_tensor(f"o{i + 1}", [c, HW], f32, kind="ExternalOutput"))

    dwcyc = cycle(["v", "ag", "av", "ag", "v", "ag", "g"])
    open_cms = {}

    with tile.TileContext(nc) as tc, ExitStack() as top:

        def popen(name, bufs=1, space="SBUF", side=None):
            cm = tc.tile_pool(name=name, bufs=bufs, space=space, side=side)
            open_cms[name] = cm
            return cm.__enter__()

        def pclose(*names):
            for n in names:
                open_cms.pop(n).__exit__(None, None, None)

        const = top.enter_context(tc.tile_pool(name="const", bufs=1))
        scrp = top.enter_context(tc.tile_pool(name="scr", bufs=2))
        smallp = top.enter_context(tc.tile_pool(name="small", bufs=4))
        statp = top.enter_context(tc.tile_pool(name="statp", bufs=1))
        p_mm = top.enter_context(tc.tile_pool(name="p_mm", bufs=2, space="PSUM"))
        p_sm = top.enter_context(tc.tile_pool(name="p_sm", bufs=2, space="PSUM"))

        ident = const.tile([P, P], f32)
        make_identity(nc, ident)
        ones = const.tile([P, P], f32)
        nc.vector.memset(ones, 1.0)
        zc = const.tile([P, 1], f32)
        nc.vector.memset(zc, 0.0)
        ec = const.tile([P, 1], f32)
        nc.vector.memset(ec, EPS)
        ident_bf = const.tile([P, P], bf16)
        make_identity(nc, ident_bf)
        ones_bf = const.tile([P, 8], bf16)
        nc.vector.memset(ones_bf, 1.0)

        # ---------- helpers ----------
        def dwconv_chain(qdst, pairs, pt, ekey):
            """qdst: [pt, 1024] output. pairs: [(apad, taps[pt,15])].

            Modes (chain-level engine assignment, balanced from profile):
              v  : DVE fused scalar_tensor_tensor MACs
              av : ACT multiply (Copy w/ per-partition scale) + DVE TT add
              ag : ACT multiply + GpSimd TT add
              g  : GpSimd broadcast-TT multiply + GpSimd TT add
            """
            qv = qdst.rearrange("p (y x) -> p y x", x=32)

            def mul_into(dst, src, w, nel):
                # dst = src * w  (w: [pt,1] per-partition scalar)
                if ekey in ("av", "ag"):
                    nc.scalar.activation(dst, src, AF.Copy, scale=w)
                elif ekey == "g":
                    nc.gpsimd.tensor_tensor(
                        dst, src, w.to_broadcast(src.shape), OP.mult)
                else:
                    nc.vector.tensor_scalar_mul(dst, src, w)

            def mac(dst, src, w, nel):
                if ekey == "v":
                    nc.vector.scalar_tensor_tensor(dst, src, w, dst, OP.mult, OP.add)
                    return
                tmp = scrp.tile([P, HW], f32, tag="mtmp", bufs=6)
                tv = tmp[:pt, : nel] if len(src.shape) == 2 else tmp[
                    :pt, 0 : nel].rearrange("p (y o) -> p y o", o=1)[:, :, 0]
                mul_into(tv, src, w, nel)
                adder = nc.vector if ekey == "av" else nc.gpsimd
                adder.tensor_tensor(dst, dst, tv, OP.add)

            first = True
            for apad, tp in pairs:
                for dy in (-1, 0, 1):
                    for dx in (-1, 0, 1):
                        off = 32 * dy + dx
                        src = apad[:pt, GL + off : GL + off + HW]
                        w = tp[:pt, (dy + 1) * 3 + (dx + 1) : (dy + 1) * 3 + (dx + 1) + 1]
                        if first:
                            mul_into(qdst, src, w, HW)
                            first = False
                        else:
                            mac(qdst, src, w, HW)
            # seam fixups (x-wraparound corrections at columns 0 and 31)
            for apad, tp in pairs:
                for dyi, dy in enumerate((-1, 0, 1)):
                    wL = apad[:pt, 32 * dy + 33 : 32 * dy + 33 + HW].rearrange(
                        "p (y x) -> p y x", x=32
                    )
                    mac(qv[:, :, 0], wL[:, :, 0], tp[:pt, 9 + dyi : 10 + dyi], 32)
                    wR = apad[:pt, 32 * dy + 66 : 32 * dy + 66 + HW].rearrange(
                        "p (y x) -> p y x", x=32
                    )
                    mac(qv[:, :, 31], wR[:, :, 0], tp[:pt, 12 + dyi : 13 + dyi], 32)

        def evict_to_apad(apad, ps, pt, ekey):
            e = nc.vector if ekey in ("v", "av") else nc.gpsimd
            e.memset(apad[:pt, 0:GL], 0.0)
            e.memset(apad[:pt, GL + HW : GL + HW + GL], 0.0)
            nc.scalar.copy(apad[:pt, GL : GL + 512], ps[:pt, 0:512])
            nc.scalar.copy(apad[:pt, GL + 512 : GL + HW], ps[:pt, 512:1024])

        def l2norm_rows(qslice, pt):
            """qslice [pt, 1024] -> divide rows by max(||row||, 1e-12)."""
            scr = scrp.tile([P, HW], f32, tag="scr")
            ss = smallp.tile([P, 1], f32, tag="ss")
            nc.scalar.activation(
                scr[:pt], qslice, AF.Square, bias=zc[:pt, 0:1],
                accum_out=ss[:pt, 0:1]
            )
            nc.scalar.activation(ss[:pt, 0:1], ss[:pt, 0:1], AF.Sqrt,
                                 bias=zc[:pt, 0:1])
            nc.vector.tensor_scalar_max(ss[:pt, 0:1], ss[:pt, 0:1], 1e-12)
            rn = smallp.tile([P, 1], f32, tag="rn")
            nc.vector.reciprocal(rn[:pt, 0:1], ss[:pt, 0:1])
            nc.scalar.activation(qslice, qslice, AF.Copy, scale=rn[:pt, 0:1])

        def instnorm_scalars(tiles, nvalid, name):
            """tiles: list of (ap [pt, w], pt). Returns (escale, ebias) [128,1]."""
            stats_ps = p_sm.tile([P, 8], f32, tag="d")
            for t, (ap, pt) in enumerate(tiles):
                stp = smallp.tile([P, 2], f32, tag="stp")
                scr = scrp.tile([P, HW], f32, tag="scr")
                w = ap.shape[-1]
                nc.scalar.activation(
                    scr[:pt, :w], ap, AF.Square, bias=zc[:pt, 0:1],
                    accum_out=stp[:pt, 1:2]
                )
                nc.scalar.activation(
                    scr[:pt, :w], ap, AF.Identity, bias=zc[:pt, 0:1],
                    accum_out=stp[:pt, 0:1]
                )
                nc.tensor.matmul(
                    stats_ps[:, 0:2], ones[:pt, :], stp[:pt, 0:2],
                    start=(t == 0), stop=(t == len(tiles) - 1),
                )
            st = statp.tile([P, 8], f32, name=f"st_{name}")
            nc.vector.tensor_copy(st[:, 0:2], stats_ps[:, 0:2])
            m_s = st[:, 2:3]
            es2 = st[:, 3:4]
            var = st[:, 4:5]
            nc.vector.tensor_scalar_mul(m_s, st[:, 0:1], SCALE / nvalid)
            nc.vector.tensor_scalar_mul(es2, st[:, 1:2], SCALE * SCALE / nvalid)
            nc.vector.tensor_tensor(var, m_s, m_s, OP.mult)
            nc.vector.tensor_tensor(var, es2, var, OP.subtract)
            sd = st[:, 5:6]
            nc.scalar.activation(sd, var, AF.Sqrt, bias=ec[:, 0:1])
            rstd = st[:, 6:7]
            nc.vector.reciprocal(rstd, sd)
            escale = statp.tile([P, 1], f32, name=f"esc_{name}")
            ebias = statp.tile([P, 1], f32, name=f"ebi_{name}")
            nc.vector.tensor_scalar_mul(escale, rstd, SCALE)
            nc.vector.tensor_tensor(ebias, m_s, rstd, OP.mult)
            nc.vector.tensor_scalar_mul(ebias, ebias, -1.0)
            return escale, ebias

        # ================= stage 1 =================
        pv = popen("pv", side="right")  # v: until ctx done
        pqk = popen("pqk")  # q,k: until transposes done
        q_sb = pqk.tile([P, NKT, HW], f32, name="q_sb")
        k_sb = pqk.tile([P, NKT, HW], f32, name="k_sb")
        v_bf = pv.tile([P, NKT, HW + 8], bf16, name="v_bf")

        pA = popen("pA")
        pw = popen("pw", bufs=2)
        pap = popen("pap", bufs=4)
        x_sb = pA.tile([P, NKT, HW], bf16, name="x_sb")
        nc.sync.dma_start(x_sb[:], x_all.rearrange("(kt p) n -> p kt n", p=P))
        taps1 = {}
        for t in "qkv":
            tt = pA.tile([P, NKT, 15], f32, name=f"taps1{t}")
            nc.sync.dma_start(tt[:], taps1d[t].rearrange("kt p f -> p kt f"))
            taps1[t] = tt

        for t in "qkv":
            wm = pw.tile([P, NKT, KVP], bf16, tag="wm")
            nc.sync.dma_start(wm[:], wT[t].rearrange("(ko p) m -> p ko m", p=P))
            for m in range(NKT):
                ps = p_mm.tile([P, 1024], f32, tag="mm")
                for kt in range(NKT):
                    nc.tensor.matmul(
                        ps[:, 0:512], r(wm[:, kt, m * P : (m + 1) * P]),
                        r(x_sb[:, kt, 0:512]),
                        start=(kt == 0), stop=(kt == NKT - 1),
                    )
                    nc.tensor.matmul(
                        ps[:, 512:1024], r(wm[:, kt, m * P : (m + 1) * P]),
                        r(x_sb[:, kt, 512:1024]),
                        start=(kt == 0), stop=(kt == NKT - 1),
                    )
                ek = next(dwcyc)
                apad = pap.tile([P, APAD_W], f32, tag="apad")
                evict_to_apad(apad, ps, P, ek)
                if t == "v":
                    vtmp = pw.tile([P, HW], f32, tag="vtmp")
                    dwconv_chain(vtmp[:, 0:HW], [(apad, taps1[t][:, m, :])], P, ek)
                    nc.scalar.copy(v_bf[:, m, 0:HW], vtmp[:, 0:HW])
                    nc.vector.memset(v_bf[:, m, HW : HW + 1], 1.0)
                else:
                    dst = q_sb if t == "q" else k_sb
                    dwconv_chain(dst[:, m, 0:HW], [(apad, taps1[t][:, m, :])], P, ek)
                    l2norm_rows(dst[:, m, 0:HW], P)
        pclose("pap", "pw", "pA")

        # ---- transposes Q,K -> n-major ----
        pT = popen("pT", side="right")
        qT = pT.tile([P, NKT, KVP], bf16, name="qT")
        kT = pT.tile([P, NKT, KVP], bf16, name="kT")
        for src, dstT in ((q_sb, qT), (k_sb, kT)):
            for j in range(NKT):
                pst = p_mm.tile([P, 1024], f32, tag="mm")
                for m in range(NKT):
                    nc.tensor.transpose(
                        pst[:, m * P : (m + 1) * P],
                        src[:, m, j * P : (j + 1) * P],
                        ident,
                    )
                nc.scalar.copy(dstT[:, j, 0:512], pst[:, 0:512])
                nc.scalar.copy(dstT[:, j, 512:1024], pst[:, 512:1024])
        pclose("pqk")

        # ---- S^T = (K^T)' @ Q^T ; tiles over d (960) ----
        pctx = popen("pctx")  # opened early for LIFO: outlives psT
        ctx = pctx.tile([P, 8, HW], bf16, name="ctx")
        peT = popen("peT")
        eT = peT.tile([P, 8, KV], bf16, name="eT")
        psT = popen("psT")
        sT = psT.tile([P, 8, KV], f32, name="sT")
        for t, (ds, pt) in enumerate(dtiles()):
            ps = p_mm.tile([P, 1024], f32, tag="mm")
            for j in range(NKT):
                lh = kT[:, j, ds : ds + pt]
                nc.tensor.matmul(
                    ps[:pt, 0:512], r(lh), r(qT[:, j, 0:512]),
                    start=(j == 0), stop=(j == NKT - 1),
                )
                nc.tensor.matmul(
                    ps[:pt, 512:KV], r(lh), r(qT[:, j, 512:KV]),
                    start=(j == 0), stop=(j == NKT - 1),
                )
            nc.scalar.copy(sT[:pt, t, 0:KV], ps[:pt, 0:KV])
        pclose("pT")

        esc1, ebi1 = instnorm_scalars(
            [(sT[:pt, t, 0:KV], pt) for t, (ds, pt) in enumerate(dtiles())],
            KV * KV, "s1",
        )
        for t, (ds, pt) in enumerate(dtiles()):
            nc.scalar.activation(
                eT[:pt, t, 0:KV], sT[:pt, t, 0:KV], AF.Exp,
                bias=ebi1[:pt, 0:1], scale=esc1[:pt, 0:1],
            )
        pclose("psT")

        # ---- ctx = (E^T)' @ [V | 1] with row-sum normalization ----
        for m, (ms, mw) in enumerate(dtiles()):
            ps = p_mm.tile([P, 1024], f32, tag="mm")
            psd = p_sm.tile([P, 8], f32, tag="d")
            for t, (ds, pt) in enumerate(dtiles()):
                lh = eT[:pt, t, ms : ms + mw]
                st_, sp_ = (t == 0), (t == 7)
                nc.tensor.matmul(ps[:mw, 0:512], lh, v_bf[:pt, t, 0:512],
                                 start=st_, stop=sp_)
                nc.tensor.matmul(ps[:mw, 512:1024], lh, v_bf[:pt, t, 512:1024],
                                 start=st_, stop=sp_)
                nc.tensor.matmul(psd[:mw, 0:1], lh, v_bf[:pt, t, HW : HW + 1],
                                 start=st_, stop=sp_)
            rd = smallp.tile([P, 1], f32, tag="rd")
            nc.vector.reciprocal(rd[:mw, 0:1], psd[:mw, 0:1])
            nc.scalar.activation(ctx[:mw, m, 0:512], ps[:mw, 0:512], AF.Copy,
                                 scale=rd[:mw, 0:1])
            nc.scalar.activation(ctx[:mw, m, 512:1024], ps[:mw, 512:1024], AF.Copy,
                                 scale=rd[:mw, 0:1])
        pclose("peT")
        pclose("pv")

        # ---- ctxT ----
        pctxT = popen("pctxT")
        ctxT = pctxT.tile([P, NKT, KV], bf16, name="ctxT")
        for j in range(NKT):
            pst = p_mm.tile([P, 1024], bf16, tag="mm")
            for m, (ms, mw) in enumerate(dtiles()):
                nc.tensor.transpose(
                    pst[:, ms : ms + mw], ctx[:mw, m, j * P : (j + 1) * P],
                    ident_bf[:mw, :mw],
                )
            nc.scalar.copy(ctxT[:, j, 0:KV], pst[:, 0:KV])

        # ================= branches: phase A =================
        pqTb = popen("pqTb")
        qTb = []
        pbA = popen("pbA", bufs=2)
        pbw = popen("pbw", bufs=2)
        pap2 = popen("pap2", bufs=4)
        for i, c in enumerate(CH):
            ct, pb = CTS[i], min(P, c)
            xb = pbA.tile([P, 4, HW], bf16, tag="xb")
            nc.sync.dma_start(
                xb[:pb, :ct, :], xb_d[i].rearrange("(ct p) n -> p ct n", p=pb)
            )
            wa = pbw.tile([P, 4, 512], bf16, tag="wab")
            nc.sync.dma_start(
                wa[:pb, :ct, :c], waT_d[i].rearrange("(kt p) m -> p kt m", p=pb)
            )
            wb = pbw.tile([P, 4, 512], bf16, tag="wab")
            nc.sync.dma_start(
                wb[:pb, :ct, :c], wbT_d[i].rearrange("(kt p) m -> p kt m", p=pb)
            )
            tpa = pbA.tile([P, 4, 15], f32, tag="tp")
            nc.sync.dma_start(tpa[:pb, :ct, :], tpa_d[i].rearrange("ct p f -> p ct f"))
            tpb = pbA.tile([P, 4, 15], f32, tag="tp")
            nc.sync.dma_start(tpb[:pb, :ct, :], tpb_d[i].rearrange("ct p f -> p ct f"))

            qb = pbA.tile([P, 4, HW], f32, tag="qb")
            for m in range(ct):
                mw = pb if ct == 1 else P
                pads = []
                for wsb in (wa, wb):
                    ps = p_mm.tile([P, 1024], f32, tag="mm")
                    for kt in range(ct):
                        nc.tensor.matmul(
                            ps[:mw, 0:512],
                            r(wsb[:pb, kt, m * P : m * P + mw]),
                            r(xb[:pb, kt, 0:512]),
                            start=(kt == 0), stop=(kt == ct - 1),
                        )
                        nc.tensor.matmul(
                            ps[:mw, 512:1024],
                            r(wsb[:pb, kt, m * P : m * P + mw]),
                            r(xb[:pb, kt, 512:1024]),
                            start=(kt == 0), stop=(kt == ct - 1),
                        )
                    ek = next(dwcyc)
                    apad = pap2.tile([P, APAD_W], f32, tag="apad")
                    evict_to_apad(apad, ps, mw, ek)
                    pads.append((apad, ek))
                ek = pads[0][1]
                dwconv_chain(
                    qb[:mw, m, 0:HW],
                    [(pads[0][0], tpa[:pb, m, :]), (pads[1][0], tpb[:pb, m, :])],
                    mw, ek,
                )
                l2norm_rows(qb[:mw, m, 0:HW], mw)

            qt = pqTb.tile([P, NKT, c], bf16, name=f"qTb{i}")
            for j in range(NKT):
                pst = p_mm.tile([P, 1024], f32, tag="mm")
                for m in range(ct):
                    mw = pb if ct == 1 else P
                    nc.tensor.transpose(
                        pst[:, m * P : m * P + mw],
                        qb[:mw, m, j * P : (j + 1) * P],
                        ident[:mw, :mw],
                    )
                nc.scalar.copy(qt[:, j, 0:c], pst[:, 0:c])
            qTb.append(qt)
        pclose("pap2", "pbw", "pbA")

        # ================= branches: phase B1 (attn^T + stats) ============
        ps2T = popen("ps2T", side="right")
        s2T, escb, ebib = [], [], []
        for i, c in enumerate(CH):
            st2 = ps2T.tile([P, 8, c], bf16, name=f"s2T{i}")
            for t, (ds, pt) in enumerate(dtiles()):
                ps = p_mm.tile([P, 1024], f32, tag="mm")
                for j in range(NKT):
                    nc.tensor.matmul(
                        ps[:pt, 0:c], r(ctxT[:, j, ds : ds + pt]),
                        r(qTb[i][:, j, 0:c]),
                        start=(j == 0), stop=(j == NKT - 1),
                    )
                nc.scalar.copy(st2[:pt, t, 0:c], ps[:pt, 0:c])
            s2T.append(st2)
            es, eb = instnorm_scalars(
                [(st2[:pt, t, 0:c], pt) for t, (ds, pt) in enumerate(dtiles())],
                KV * c, f"b{i}",
            )
            escb.append(es)
            ebib.append(eb)
        pclose("pqTb", "pctxT")

        # ================= branches: phase B2 (exp, o, proj, out) =========
        for i, c in enumerate(CH):
            ct, pb = CTS[i], min(P, c)
            st2 = s2T[i]
            for t, (ds, pt) in enumerate(dtiles()):
                nc.scalar.activation(
                    st2[:pt, t, 0:c], st2[:pt, t, 0:c], AF.Exp,
                    bias=ebib[i][:pt, 0:1], scale=escb[i][:pt, 0:1],
                )
            with tc.tile_pool(name=f"pb2_{i}", bufs=1) as pb2:
                ob = pb2.tile([pb, ct, HW], bf16, name=f"ob{i}")
                for m in range(ct):
                    mw = pb if ct == 1 else P
                    ps = p_mm.tile([P, 1024], f32, tag="mm")
                    psd = p_sm.tile([P, 8], f32, tag="d")
                    for t, (ds, pt) in enumerate(dtiles()):
                        lh = st2[:pt, t, m * P : m * P + mw]
                        st_, sp_ = (t == 0), (t == 7)
                        nc.tensor.matmul(ps[:mw, 0:512], r(lh),
                                         r(ctx[:pt, t, 0:512]),
                                         start=st_, stop=sp_)
                        nc.tensor.matmul(ps[:mw, 512:1024], r(lh),
                                         r(ctx[:pt, t, 512:1024]),
                                         start=st_, stop=sp_)
                        nc.tensor.matmul(psd[:mw, 0:1], lh, ones_bf[:pt, 0:1],
                                         start=st_, stop=sp_)
                    rd = smallp.tile([P, 1], f32, tag="rd")
                    nc.vector.reciprocal(rd[:mw, 0:1], psd[:mw, 0:1])
                    nc.scalar.activation(ob[:mw, m, 0:512], ps[:mw, 0:512],
                                         AF.Copy, scale=rd[:mw, 0:1])
                    nc.scalar.activation(ob[:mw, m, 512:1024], ps[:mw, 512:1024],
                                         AF.Copy, scale=rd[:mw, 0:1])
                # proj
                wp = pb2.tile([pb, ct, c], bf16, name=f"wp{i}")
                nc.sync.dma_start(
                    wp[:], wpT_d[i].rearrange("(kt p) m -> p kt m", p=pb)
                )
                outb = pb2.tile([pb, ct, HW], f32, name=f"outb{i}")
                for m in range(ct):
                    mw = pb if ct == 1 else P
                    ps = p_mm.tile([P, 1024], f32, tag="mm")
                    for kt in range(ct):
                        nc.tensor.matmul(
                            ps[:mw, 0:512], r(wp[:pb, kt, m * P : m * P + mw]),
                            r(ob[:pb, kt, 0:512]),
                            start=(kt == 0), stop=(kt == ct - 1),
                        )
                        nc.tensor.matmul(
                            ps[:mw, 512:1024], r(wp[:pb, kt, m * P : m * P + mw]),
                            r(ob[:pb, kt, 512:1024]),
                            start=(kt == 0), stop=(kt == ct - 1),
                        )
                    nc.scalar.copy(outb[:mw, m, 0:512], ps[:mw, 0:512])
                    nc.scalar.copy(outb[:mw, m, 512:1024], ps[:mw, 512:1024])
                nc.sync.dma_start(
                    out_d[i].rearrange("(ct p) n -> p ct n", p=pb), outb[:]
                )
        pclose("ps2T", "pctx")

    nc.compile()
    return nc


def _prep_taps(w, c):
    """w: [c, 3, 3] -> [c, 15]: 9 taps + 3 negated dx=-1 + 3 negated dx=+1."""
    t = np.zeros((c, 15), np.float32)
    t[:, 0:9] = w.reshape(c, 9)
    for dyi in range(3):
        t[:, 9 + dyi] = -w[:, dyi, 0]
        t[:, 12 + dyi] = -w[:, dyi, 2]
    return t


def _host_prep(inputs):
    """Build the shared (weight) tensors + per-core input maps."""
    import ml_dtypes

    f = np.float32
    bf = ml_dtypes.bfloat16
    shared = {}
    for t, wk, dk in (("q", "w_mheadq", "w_qc"), ("k", "w_mheadk", "w_kc"),
                      ("v", "w_mheadv", "w_vc")):
        w = np.zeros((KVP, KVP), bf)
        w[:KV, :KV] = np.asarray(inputs[wk]).astype(f).T.astype(bf)
        shared[f"wT_{t}"] = w
        tp = np.zeros((KVP, 15), f)
        tp[:KV] = _prep_taps(np.asarray(inputs[dk])[:, 0].astype(f), KV)
        shared[f"taps_{t}"] = np.ascontiguousarray(tp.reshape(NKT, P, 15))
    for i, c in enumerate(CH):
        ct, pb = CTS[i], min(P, c)
        wmh = np.asarray(inputs[f"w_mhead{i + 1}"]).astype(f)
        idx = np.arange(c)
        wa = wmh[2 * (idx // 2)]
        wb = wmh[2 * (idx // 2) + 1]
        shared[f"waT{i + 1}"] = np.ascontiguousarray(wa.T).astype(bf)
        shared[f"wbT{i + 1}"] = np.ascontiguousarray(wb.T).astype(bf)
        wq = np.asarray(inputs[f"w_q{i + 1}"]).astype(f)  # [c, 2, 3, 3]
        shared[f"tapsa{i + 1}"] = np.ascontiguousarray(
            _prep_taps(wq[:, 0], c).reshape(ct, pb, 15))
        shared[f"tapsb{i + 1}"] = np.ascontiguousarray(
            _prep_taps(wq[:, 1], c).reshape(ct, pb, 15))
        shared[f"wpT{i + 1}"] = np.ascontiguousarray(
            np.asarray(inputs[f"w_proj{i + 1}"]).astype(f).T).astype(bf)

    in_maps = []
    B = np.asarray(inputs["emb_all"]).shape[0]
    for s in range(B):
        m = dict(shared)
        xa = np.zeros((KVP, HW), bf)
        xa[:KV] = np.asarray(inputs["emb_all"])[s].reshape(KV, HW).astype(bf)
        m["x_all"] = xa
        for i, c in enumerate(CH):
            m[f"x{i + 1}"] = np.ascontiguousarray(
                np.asarray(inputs[f"emb{i + 1}"])[s].reshape(c, HW).astype(f)
            ).astype(bf)
        in_maps.append(m)
    return in_maps


def kernel(**inputs):
    from concourse.bass_utils import run_bass_kernel_spmd

    if "nc" not in _CACHE:
        _CACHE["nc"] = _build_nc()
    nc = _CACHE["nc"]

    in_maps = _host_prep(inputs)
    trace = os.environ.get("KERNEL_TRACE", "0") == "1"
    kw = {}
    if trace:
        kw = dict(trace=True, trace_cores=[0])
    res = run_bass_kernel_spmd(nc, in_maps, core_ids=list(range(8)), **kw)
    if trace and res.exec_time_ns is not None:
        print(f"HW exec time: {res.exec_time_ns} ns")
        if res.instructions_and_trace is not None:
            print("trace:", res.instructions_and_trace[1])
        _CACHE["last_result"] = res

    B = len(in_maps)
    outs = []
    for i, c in enumerate(CH):
        o = np.stack([res.results[s][f"o{i + 1}"] for s in range(B)])
        outs.append(o.reshape(B, c, 32, 32).astype(np.float32))
    return tuple(outs)


# revision 23
# speedup vs baseline: 2.4938x; 1.0197x over previous
"""Trainium2 Bass kernel for nn_Attention_org_29961691856973.

Sharding: pure data-parallel over batch (B=8 -> 8 NeuronCores, 1 sample each).
All weights replicated; no collectives.

Per-core pipeline (channel-major layout: channels on partitions, hw=1024 free):
  stage 1: A_t = W_t @ X (f32r matmuls), depthwise 3x3 via fused MAC chains on
           DVE/GpSimd (flat layout + zero guards + seam fixups), l2norm rows,
           PE transposes Q,K -> n-major, S^T = K^T' Q^T, instance-norm stats via
           ACT accum + ones-matmul partition reduction, exp fused w/ scale/bias,
           ctx = (E^T)^T [V | 1] with softmax row-sums from the ones column.
  stage 2 (x4 branches): grouped conv folded into duplicated 1x1 weights
           (host-prepped), same dwconv machinery, attn^T = ctxT' qT, instnorm +
           softmax via the same transposed-E trick, o = (E2^T)^T ctx, proj.
"""

import math
import os

import numpy as np

P = 128
HW = 1024
KV = 960
KVP = 1024
NKT = KVP // P  # 8
CH = [64, 128, 256, 512]
CTS = [1, 1, 2, 4]  # channel tiles per branch
SCALE = 1.0 / math.sqrt(KV)
EPS = 1e-5
APAD_W = 1128  # 34 guard + 1024 + 34 guard + slack (max AP window start 98+1024)
GL = 34  # left guard size / interior start

# 960 = 7*128 + 64 tiling helper
def dtiles():
    return [(t * P, P if t < 7 else 64) for t in range(8)]


_CACHE = {}


def _build_nc():
    import concourse.bass as bass
    import concourse.mybir as mybir
    import concourse.tile as tile
    from concourse import bacc
    from concourse.masks import make_identity
    from contextlib import ExitStack
    from itertools import cycle

    f32 = mybir.dt.float32
    bf16 = mybir.dt.bfloat16
    AF = mybir.ActivationFunctionType
    OP = mybir.AluOpType

    nc = bacc.Bacc(None, target_bir_lowering=False)

    def r(ap):
        return ap

    # ------------- DRAM I/O -------------
    x_all = nc.dram_tensor("x_all", [KVP, HW], bf16, kind="ExternalInput")
    wT = {
        t: nc.dram_tensor(f"wT_{t}", [KVP, KVP], bf16, kind="ExternalInput")
        for t in "qkv"
    }
    taps1d = {
        t: nc.dram_tensor(f"taps_{t}", [NKT, P, 15], f32, kind="ExternalInput")
        for t in "qkv"
    }
    xb_d, waT_d, wbT_d, tpa_d, tpb_d, wpT_d, out_d = [], [], [], [], [], [], []
    for i, c in enumerate(CH):
        ct = CTS[i]
        pb = min(P, c)
        xb_d.append(nc.dram_tensor(f"x{i + 1}", [c, HW], bf16, kind="ExternalInput"))
        waT_d.append(nc.dram_tensor(f"waT{i + 1}", [c, c], bf16, kind="ExternalInput"))
        wbT_d.append(nc.dram_tensor(f"wbT{i + 1}", [c, c], bf16, kind="ExternalInput"))
        tpa_d.append(
            nc.dram_tensor(f"tapsa{i + 1}", [ct, pb, 15], f32, kind="ExternalInput")
        )
        tpb_d.append(
            nc.dram_tensor(f"tapsb{i + 1}", [ct, pb, 15], f32, kind="ExternalInput")
        )
        wpT_d.append(nc.dram_tensor(f"wpT{i + 1}", [c, c], bf16, kind="ExternalInput"))
        out_d.append(nc.dram_tensor(f"o{i + 1}", [c, HW], f32, kind="ExternalOutput"))

    dwcyc = cycle(["v", "ag", "av", "ag", "v", "ag", "g"])
    open_cms = {}

    with tile.TileContext(nc) as tc, ExitStack() as top:

        def popen(name, bufs=1, space="SBUF", side=None):
            cm = tc.tile_pool(name=name, bufs=bufs, space=space, side=side)
            open_cms[name] = cm
            return cm.__enter__()

        def pclose(*names):
            for n in names:
                open_cms.pop(n).__exit__(None, None, None)

        const = top.enter_context(tc.tile_pool(name="const", bufs=1))
        scrp = top.enter_context(tc.tile_pool(name="scr", bufs=2))
        smallp = top.enter_context(tc.tile_pool(name="small", bufs=4))
        statp = top.enter_context(tc.tile_pool(name="statp", bufs=1))
        p_mm = top.enter_context(tc.tile_pool(name="p_mm", bufs=2, space="PSUM"))
        p_sm = top.enter_context(tc.tile_pool(name="p_sm", bufs=2, space="PSUM"))

        ident = const.tile([P, P], f32)
        make_identity(nc, ident)
        ones = const.tile([P, P], f32)
        nc.vector.memset(ones, 1.0)
        zc = const.tile([P, 1], f32)
        nc.vector.memset(zc, 0.0)
        ec = const.tile([P, 1], f32)
        nc.vector.memset(ec, EPS)
        ident_bf = const.tile([P, P], bf16)
        make_identity(nc, ident_bf)
        ones_bf = const.tile([P, 8], bf16)
        nc.vector.memset(ones_bf, 1.0)

        # ---------- helpers ----------
        def dwconv_chain(qdst, pairs, pt, ekey):
            """qdst: [pt, 1024] output. pairs: [(apad, taps[pt,15])].

            Modes (chain-level engine assignment, balanced from profile):
              v  : DVE fused scalar_tensor_tensor MACs
              av : ACT multiply (Copy w/ per-partition scale) + DVE TT add
              ag : ACT multiply + GpSimd TT add
              g  : GpSimd broadcast-TT multiply + GpSimd TT add
            """
            qv = qdst.rearrange("p (y x) -> p y x", x=32)

            def mul_into(dst, src, w, nel):
                # dst = src * w  (w: [pt,1] per-partition scalar)
                if ekey in ("av", "ag"):
                    nc.scalar.activation(dst, src, AF.Copy, scale=w)
                elif ekey == "g":
                    nc.gpsimd.tensor_tensor(
                        dst, src, w.to_broadcast(src.shape), OP.mult)
                else:
                    nc.vector.tensor_scalar_mul(dst, src, w)

            def mac(dst, src, w, nel):
                if ekey == "v":
                    nc.vector.scalar_tensor_tensor(dst, src, w, dst, OP.mult, OP.add)
                    return
                tmp = scrp.tile([P, HW], f32, tag="mtmp", bufs=5)
                tv = tmp[:pt, : nel] if len(src.shape) == 2 else tmp[
                    :pt, 0 : nel].rearrange("p (y o) -> p y o", o=1)[:, :, 0]
                mul_into(tv, src, w, nel)
                adder = nc.vector if ekey == "av" else nc.gpsimd
                adder.tensor_tensor(dst, dst, tv, OP.add)

            first = True
            for apad, tp in pairs:
                for dy in (-1, 0, 1):
                    for dx in (-1, 0, 1):
                        off = 32 * dy + dx
                        src = apad[:pt, GL + off : GL + off + HW]
                        w = tp[:pt, (dy + 1) * 3 + (dx + 1) : (dy + 1) * 3 + (dx + 1) + 1]
                        if first:
                            mul_into(qdst, src, w, HW)
                            first = False
                        else:
                            mac(qdst, src, w, HW)
            # seam fixups (x-wraparound corrections at columns 0 and 31)
            for apad, tp in pairs:
                for dyi, dy in enumerate((-1, 0, 1)):
                    wL = apad[:pt, 32 * dy + 33 : 32 * dy + 33 + HW].rearrange(
                        "p (y x) -> p y x", x=32
                    )
                    mac(qv[:, :, 0], wL[:, :, 0], tp[:pt, 9 + dyi : 10 + dyi], 32)
                    wR = apad[:pt, 32 * dy + 66 : 32 * dy + 66 + HW].rearrange(
                        "p (y x) -> p y x", x=32
                    )
                    mac(qv[:, :, 31], wR[:, :, 0], tp[:pt, 12 + dyi : 13 + dyi], 32)

        def evict_to_apad(apad, ps, pt, ekey):
            e = nc.vector if ekey in ("v", "av") else nc.gpsimd
            e.memset(apad[:pt, 0:GL], 0.0)
            e.memset(apad[:pt, GL + HW : GL + HW + GL], 0.0)
            nc.scalar.copy(apad[:pt, GL : GL + 512], ps[:pt, 0:512])
            nc.scalar.copy(apad[:pt, GL + 512 : GL + HW], ps[:pt, 512:1024])

        def l2norm_rows(qslice, pt):
            """qslice [pt, 1024] -> divide rows by max(||row||, 1e-12)."""
            scr = scrp.tile([P, HW], f32, tag="scr")
            ss = smallp.tile([P, 1], f32, tag="ss")
            nc.scalar.activation(
                scr[:pt], qslice, AF.Square, bias=zc[:pt, 0:1],
                accum_out=ss[:pt, 0:1]
            )
            nc.scalar.activation(ss[:pt, 0:1], ss[:pt, 0:1], AF.Sqrt,
                                 bias=zc[:pt, 0:1])
            nc.vector.tensor_scalar_max(ss[:pt, 0:1], ss[:pt, 0:1], 1e-12)
            rn = smallp.tile([P, 1], f32, tag="rn")
            nc.vector.reciprocal(rn[:pt, 0:1], ss[:pt, 0:1])
            nc.scalar.activation(qslice, qslice, AF.Copy, scale=rn[:pt, 0:1])

        def instnorm_scalars(tiles, nvalid, name):
            """tiles: list of (ap [pt, w], pt). Returns (escale, ebias) [128,1]."""
            stats_ps = p_sm.tile([P, 8], f32, tag="d")
            for t, (ap, pt) in enumerate(tiles):
                stp = smallp.tile([P, 2], f32, tag="stp")
                scr = scrp.tile([P, HW], f32, tag="scr")
                w = ap.shape[-1]
                nc.scalar.activation(
                    scr[:pt, :w], ap, AF.Square, bias=zc[:pt, 0:1],
                    accum_out=stp[:pt, 1:2]
                )
                nc.vector.reduce_sum(stp[:pt, 0:1], ap,
                                     axis=mybir.AxisListType.X)
                nc.tensor.matmul(
                    stats_ps[:, 0:2], ones[:pt, :], stp[:pt, 0:2],
                    start=(t == 0), stop=(t == len(tiles) - 1),
                )
            st = statp.tile([P, 8], f32, name=f"st_{name}")
            nc.vector.tensor_copy(st[:, 0:2], stats_ps[:, 0:2])
            m_s = st[:, 2:3]
            es2 = st[:, 3:4]
            var = st[:, 4:5]
            nc.vector.tensor_scalar_mul(m_s, st[:, 0:1], SCALE / nvalid)
            nc.vector.tensor_scalar_mul(es2, st[:, 1:2], SCALE * SCALE / nvalid)
            nc.vector.tensor_tensor(var, m_s, m_s, OP.mult)
            nc.vector.tensor_tensor(var, es2, var, OP.subtract)
            sd = st[:, 5:6]
            nc.scalar.activation(sd, var, AF.Sqrt, bias=ec[:, 0:1])
            rstd = st[:, 6:7]
            nc.vector.reciprocal(rstd, sd)
            escale = statp.tile([P, 1], f32, name=f"esc_{name}")
            ebias = statp.tile([P, 1], f32, name=f"ebi_{name}")
            nc.vector.tensor_scalar_mul(escale, rstd, SCALE)
            nc.vector.tensor_tensor(ebias, m_s, rstd, OP.mult)
            nc.vector.tensor_scalar_mul(ebias, ebias, -1.0)
            return escale, ebias

        # ================= stage 1 =================
        pv = popen("pv", side="right")  # v: until ctx done
        pqTb = popen("pqTb")  # branch qT tiles: closed at the very end (LIFO)
        pqk = popen("pqk")  # q,k: until transposes done
        q_sb = pqk.tile([P, NKT, HW], f32, name="q_sb")
        k_sb = pqk.tile([P, NKT, HW], f32, name="k_sb")
        v_bf = pv.tile([P, NKT, HW + 8], bf16, name="v_bf")

        pA = popen("pA")
        pw = popen("pw", bufs=2)
        pap = popen("pap", bufs=4)
        x_sb = pA.tile([P, NKT, HW], bf16, name="x_sb")
        nc.sync.dma_start(x_sb[:], x_all.rearrange("(kt p) n -> p kt n", p=P))
        taps1 = {}
        for t in "qkv":
            tt = pA.tile([P, NKT, 15], f32, name=f"taps1{t}")
            nc.sync.dma_start(tt[:], taps1d[t].rearrange("kt p f -> p kt f"))
            taps1[t] = tt

        for t in "qkv":
            wm = pw.tile([P, NKT, KVP], bf16, tag="wm")
            nc.sync.dma_start(wm[:], wT[t].rearrange("(ko p) m -> p ko m", p=P))
            for m in range(NKT):
                ps = p_mm.tile([P, 1024], f32, tag="mm")
                for kt in range(NKT):
                    nc.tensor.matmul(
                        ps[:, 0:512], r(wm[:, kt, m * P : (m + 1) * P]),
                        r(x_sb[:, kt, 0:512]),
                        start=(kt == 0), stop=(kt == NKT - 1),
                    )
                    nc.tensor.matmul(
                        ps[:, 512:1024], r(wm[:, kt, m * P : (m + 1) * P]),
                        r(x_sb[:, kt, 512:1024]),
                        start=(kt == 0), stop=(kt == NKT - 1),
                    )
                ek = next(dwcyc)
                apad = pap.tile([P, APAD_W], f32, tag="apad")
                evict_to_apad(apad, ps, P, ek)
                if t == "v":
                    vtmp = pw.tile([P, HW], f32, tag="vtmp")
                    dwconv_chain(vtmp[:, 0:HW], [(apad, taps1[t][:, m, :])], P, ek)
                    nc.scalar.copy(v_bf[:, m, 0:HW], vtmp[:, 0:HW])
                    nc.vector.memset(v_bf[:, m, HW : HW + 1], 1.0)
                else:
                    dst = q_sb if t == "q" else k_sb
                    dwconv_chain(dst[:, m, 0:HW], [(apad, taps1[t][:, m, :])], P, ek)
                    l2norm_rows(dst[:, m, 0:HW], P)
        pclose("pap", "pw", "pA")

        # ================= branches: phase A =================
        qTb = []
        pbA = popen("pbA", bufs=2)
        pbw = popen("pbw", bufs=2)
        pap2 = popen("pap2", bufs=3)
        for i, c in enumerate(CH):
            ct, pb = CTS[i], min(P, c)
            xb = pbA.tile([P, 4, HW], bf16, tag="xb")
            nc.sync.dma_start(
                xb[:pb, :ct, :], xb_d[i].rearrange("(ct p) n -> p ct n", p=pb)
            )
            wa = pbw.tile([P, 4, 512], bf16, tag="wab")
            nc.sync.dma_start(
                wa[:pb, :ct, :c], waT_d[i].rearrange("(kt p) m -> p kt m", p=pb)
            )
            wb = pbw.tile([P, 4, 512], bf16, tag="wab")
            nc.sync.dma_start(
                wb[:pb, :ct, :c], wbT_d[i].rearrange("(kt p) m -> p kt m", p=pb)
            )
            tpa = pbA.tile([P, 4, 15], f32, tag="tp")
            nc.sync.dma_start(tpa[:pb, :ct, :], tpa_d[i].rearrange("ct p f -> p ct f"))
            tpb = pbA.tile([P, 4, 15], f32, tag="tp")
            nc.sync.dma_start(tpb[:pb, :ct, :], tpb_d[i].rearrange("ct p f -> p ct f"))

            qb = pbA.tile([P, 4, HW], f32, tag="qb")
            for m in range(ct):
                mw = pb if ct == 1 else P
                pads = []
                for wsb in (wa, wb):
                    ps = p_mm.tile([P, 1024], f32, tag="mm")
                    for kt in range(ct):
                        nc.tensor.matmul(
                            ps[:mw, 0:512],
                            r(wsb[:pb, kt, m * P : m * P + mw]),
                            r(xb[:pb, kt, 0:512]),
                            start=(kt == 0), stop=(kt == ct - 1),
                        )
                        nc.tensor.matmul(
                            ps[:mw, 512:1024],
                            r(wsb[:pb, kt, m * P : m * P + mw]),
                            r(xb[:pb, kt, 512:1024]),
                            start=(kt == 0), stop=(kt == ct - 1),
                        )
                    ek = next(dwcyc)
                    apad = pap2.tile([P, APAD_W], f32, tag="apad")
                    evict_to_apad(apad, ps, mw, ek)
                    pads.append((apad, ek))
                ek = pads[0][1]
                dwconv_chain(
                    qb[:mw, m, 0:HW],
                    [(pads[0][0], tpa[:pb, m, :]), (pads[1][0], tpb[:pb, m, :])],
                    mw, ek,
                )
                l2norm_rows(qb[:mw, m, 0:HW], mw)

            qt = pqTb.tile([P, NKT, c], bf16, name=f"qTb{i}")
            for j in range(NKT):
                pst = p_mm.tile([P, 1024], f32, tag="mm")
                for m in range(ct):
                    mw = pb if ct == 1 else P
                    nc.tensor.transpose(
                        pst[:, m * P : m * P + mw],
                        qb[:mw, m, j * P : (j + 1) * P],
                        ident[:mw, :mw],
                    )
                nc.vector.tensor_copy(qt[:, j, 0:c], pst[:, 0:c])
            qTb.append(qt)
        pclose("pap2", "pbw", "pbA")


        # ---- transposes Q,K -> n-major ----
        pT = popen("pT", side="right")
        qT = pT.tile([P, NKT, KVP], bf16, name="qT")
        kT = pT.tile([P, NKT, KVP], bf16, name="kT")
        for src, dstT in ((q_sb, qT), (k_sb, kT)):
            for j in range(NKT):
                pst = p_mm.tile([P, 1024], f32, tag="mm")
                for m in range(NKT):
                    nc.tensor.transpose(
                        pst[:, m * P : (m + 1) * P],
                        src[:, m, j * P : (j + 1) * P],
                        ident,
                    )
                nc.vector.tensor_copy(dstT[:, j, 0:512], pst[:, 0:512])
                nc.vector.tensor_copy(dstT[:, j, 512:1024], pst[:, 512:1024])
        pclose("pqk")

        # ---- S^T = (K^T)' @ Q^T ; tiles over d (960) ----
        pctx = popen("pctx")  # opened early for LIFO: outlives psT
        ctx = pctx.tile([P, 8, HW], bf16, name="ctx")
        peT = popen("peT")
        eT = peT.tile([P, 8, KV], bf16, name="eT")
        psT = popen("psT")
        sT = psT.tile([P, 8, KV], f32, name="sT")
        for t, (ds, pt) in enumerate(dtiles()):
            ps = p_mm.tile([P, 1024], f32, tag="mm")
            for j in range(NKT):
                lh = kT[:, j, ds : ds + pt]
                nc.tensor.matmul(
                    ps[:pt, 0:512], r(lh), r(qT[:, j, 0:512]),
                    start=(j == 0), stop=(j == NKT - 1),
                )
                nc.tensor.matmul(
                    ps[:pt, 512:KV], r(lh), r(qT[:, j, 512:KV]),
                    start=(j == 0), stop=(j == NKT - 1),
                )
            nc.vector.tensor_copy(sT[:pt, t, 0:KV], ps[:pt, 0:KV])
        pclose("pT")

        esc1, ebi1 = instnorm_scalars(
            [(sT[:pt, t, 0:KV], pt) for t, (ds, pt) in enumerate(dtiles())],
            KV * KV, "s1",
        )
        for t, (ds, pt) in enumerate(dtiles()):
            nc.scalar.activation(
                eT[:pt, t, 0:KV], sT[:pt, t, 0:KV], AF.Exp,
                bias=ebi1[:pt, 0:1], scale=esc1[:pt, 0:1],
            )
        pclose("psT")

        # ---- ctx = (E^T)' @ [V | 1] with row-sum normalization ----
        for m, (ms, mw) in enumerate(dtiles()):
            ps = p_mm.tile([P, 1024], f32, tag="mm")
            psd = p_sm.tile([P, 8], f32, tag="d")
            for t, (ds, pt) in enumerate(dtiles()):
                lh = eT[:pt, t, ms : ms + mw]
                st_, sp_ = (t == 0), (t == 7)
                nc.tensor.matmul(ps[:mw, 0:512], lh, v_bf[:pt, t, 0:512],
                                 start=st_, stop=sp_)
                nc.tensor.matmul(ps[:mw, 512:1024], lh, v_bf[:pt, t, 512:1024],
                                 start=st_, stop=sp_)
                nc.tensor.matmul(psd[:mw, 0:1], lh, v_bf[:pt, t, HW : HW + 1],
                                 start=st_, stop=sp_)
            rd = smallp.tile([P, 1], f32, tag="rd")
            nc.vector.reciprocal(rd[:mw, 0:1], psd[:mw, 0:1])
            nc.vector.tensor_scalar_mul(ctx[:mw, m, 0:512], ps[:mw, 0:512],
                                        rd[:mw, 0:1])
            nc.vector.tensor_scalar_mul(ctx[:mw, m, 512:1024], ps[:mw, 512:1024],
                                        rd[:mw, 0:1])
        pclose("peT")
        pclose("pv")

        # ---- ctxT ----
        pctxT = popen("pctxT")
        ctxT = pctxT.tile([P, NKT, KV], bf16, name="ctxT")
        for j in range(NKT):
            pst = p_mm.tile([P, 1024], bf16, tag="mm")
            for m, (ms, mw) in enumerate(dtiles()):
                nc.tensor.transpose(
                    pst[:, ms : ms + mw], ctx[:mw, m, j * P : (j + 1) * P],
                    ident_bf[:mw, :mw],
                )
            nc.vector.tensor_copy(ctxT[:, j, 0:KV], pst[:, 0:KV])

        # ================= branches: phase B1 (attn^T + stats) ============
        ps2T = popen("ps2T", side="right")
        s2T, escb, ebib = [], [], []
        for i, c in enumerate(CH):
            st2 = ps2T.tile([P, 8, c], bf16, name=f"s2T{i}")
            for t, (ds, pt) in enumerate(dtiles()):
                ps = p_mm.tile([P, 1024], f32, tag="mm")
                for j in range(NKT):
                    nc.tensor.matmul(
                        ps[:pt, 0:c], r(ctxT[:, j, ds : ds + pt]),
                        r(qTb[i][:, j, 0:c]),
                        start=(j == 0), stop=(j == NKT - 1),
                    )
                nc.vector.tensor_copy(st2[:pt, t, 0:c], ps[:pt, 0:c])
            s2T.append(st2)
            es, eb = instnorm_scalars(
                [(st2[:pt, t, 0:c], pt) for t, (ds, pt) in enumerate(dtiles())],
                KV * c, f"b{i}",
            )
            escb.append(es)
            ebib.append(eb)
        pclose("pctxT")

        # ================= branches: phase B2 (exp, o, proj, out) =========
        for i, c in enumerate(CH):
            ct, pb = CTS[i], min(P, c)
            st2 = s2T[i]
            for t, (ds, pt) in enumerate(dtiles()):
                nc.scalar.activation(
                    st2[:pt, t, 0:c], st2[:pt, t, 0:c], AF.Exp,
                    bias=ebib[i][:pt, 0:1], scale=escb[i][:pt, 0:1],
                )
            with tc.tile_pool(name=f"pb2_{i}", bufs=1) as pb2:
                ob = pb2.tile([pb, ct, HW], bf16, name=f"ob{i}")
                for m in range(ct):
                    mw = pb if ct == 1 else P
                    ps = p_mm.tile([P, 1024], f32, tag="mm")
                    psd = p_sm.tile([P, 8], f32, tag="d")
                    for t, (ds, pt) in enumerate(dtiles()):
                        lh = st2[:pt, t, m * P : m * P + mw]
                        st_, sp_ = (t == 0), (t == 7)
                        nc.tensor.matmul(ps[:mw, 0:512], r(lh),
                                         r(ctx[:pt, t, 0:512]),
                                         start=st_, stop=sp_)
                        nc.tensor.matmul(ps[:mw, 512:1024], r(lh),
                                         r(ctx[:pt, t, 512:1024]),
                                         start=st_, stop=sp_)
                        nc.tensor.matmul(psd[:mw, 0:1], lh, ones_bf[:pt, 0:1],
                                         start=st_, stop=sp_)
                    rd = smallp.tile([P, 1], f32, tag="rd")
                    nc.vector.reciprocal(rd[:mw, 0:1], psd[:mw, 0:1])
                    nc.vector.tensor_scalar_mul(ob[:mw, m, 0:512], ps[:mw, 0:512],
                                                rd[:mw, 0:1])
                    nc.vector.tensor_scalar_mul(ob[:mw, m, 512:1024],
                                                ps[:mw, 512:1024], rd[:mw, 0:1])
                # proj
                wp = pb2.tile([pb, ct, c], bf16, name=f"wp{i}")
                nc.sync.dma_start(
                    wp[:], wpT_d[i].rearrange("(kt p) m -> p kt m", p=pb)
                )
                outb = pb2.tile([pb, ct, HW], f32, name=f"outb{i}")
                for m in range(ct):
                    mw = pb if ct == 1 else P
                    ps = p_mm.tile([P, 1024], f32, tag="mm")
                    for kt in range(ct):
                        nc.tensor.matmul(
                            ps[:mw, 0:512], r(wp[:pb, kt, m * P : m * P + mw]),
                            r(ob[:pb, kt, 0:512]),
                            start=(kt == 0), stop=(kt == ct - 1),
                        )
                        nc.tensor.matmul(
                            ps[:mw, 512:1024], r(wp[:pb, kt, m * P : m * P + mw]),
                            r(ob[:pb, kt, 512:1024]),
                            start=(kt == 0), stop=(kt == ct - 1),
                        )
                    nc.vector.tensor_copy(outb[:mw, m, 0:512], ps[:mw, 0:512])
                    nc.vector.tensor_copy(outb[:mw, m, 512:1024],
                                          ps[:mw, 512:1024])
                nc.sync.dma_start(
                    out_d[i].rearrange("(ct p) n -> p ct n", p=pb), outb[:]
                )
        pclose("ps2T", "pctx", "pqTb")

    nc.compile()
    return nc


def _prep_taps(w, c):
    """w: [c, 3, 3] -> [c, 15]: 9 taps + 3 negated dx=-1 + 3 negated dx=+1."""
    t = np.zeros((c, 15), np.float32)
    t[:, 0:9] = w.reshape(c, 9)
    for dyi in range(3):
        t[:, 9 + dyi] = -w[:, dyi, 0]
        t[:, 12 + dyi] = -w[:, dyi, 2]
    return t


def _host_prep(inputs):
    """Build the shared (weight) tensors + per-core input maps."""
    import ml_dtypes

    f = np.float32
    bf = ml_dtypes.bfloat16
    shared = {}
    for t, wk, dk in (("q", "w_mheadq", "w_qc"), ("k", "w_mheadk", "w_kc"),
                      ("v", "w_mheadv", "w_vc")):
        w = np.zeros((KVP, KVP), bf)
        w[:KV, :KV] = np.asarray(inputs[wk]).astype(f).T.astype(bf)
        shared[f"wT_{t}"] = w
        tp = np.zeros((KVP, 15), f)
        tp[:KV] = _prep_taps(np.asarray(inputs[dk])[:, 0].astype(f), KV)
        shared[f"taps_{t}"] = np.ascontiguousarray(tp.reshape(NKT, P, 15))
    for i, c in enumerate(CH):
        ct, pb = CTS[i], min(P, c)
        wmh = np.asarray(inputs[f"w_mhead{i + 1}"]).astype(f)
        idx = np.arange(c)
        wa = wmh[2 * (idx // 2)]
        wb = wmh[2 * (idx // 2) + 1]
        shared[f"waT{i + 1}"] = np.ascontiguousarray(wa.T).astype(bf)
        shared[f"wbT{i + 1}"] = np.ascontiguousarray(wb.T).astype(bf)
        wq = np.asarray(inputs[f"w_q{i + 1}"]).astype(f)  # [c, 2, 3, 3]
        shared[f"tapsa{i + 1}"] = np.ascontiguousarray(
            _prep_taps(wq[:, 0], c).reshape(ct, pb, 15))
        shared[f"tapsb{i + 1}"] = np.ascontiguousarray(
            _prep_taps(wq[:, 1], c).reshape(ct, pb, 15))
        shared[f"wpT{i + 1}"] = np.ascontiguousarray(
            np.asarray(inputs[f"w_proj{i + 1}"]).astype(f).T).astype(bf)

    in_maps = []
    B = np.asarray(inputs["emb_all"]).shape[0]
    for s in range(B):
        m = dict(shared)
        xa = np.zeros((KVP, HW), bf)
        xa[:KV] = np.asarray(inputs["emb_all"])[s].reshape(KV, HW).astype(bf)
        m["x_all"] = xa
        for i, c in enumerate(CH):
            m[f"x{i + 1}"] = np.ascontiguousarray(
                np.asarray(inputs[f"emb{i + 1}"])[s].reshape(c, HW).astype(f)
            ).astype(bf)
        in_maps.append(m)
    return in_maps


def kernel(**inputs):
    from concourse.bass_utils import run_bass_kernel_spmd

    if "nc" not in _CACHE:
        _CACHE["nc"] = _build_nc()
    nc = _CACHE["nc"]

    in_maps = _host_prep(inputs)
    trace = os.environ.get("KERNEL_TRACE", "0") == "1"
    kw = {}
    if trace:
        kw = dict(trace=True, trace_cores=[0])
    res = run_bass_kernel_spmd(nc, in_maps, core_ids=list(range(8)), **kw)
    if trace and res.exec_time_ns is not None:
        print(f"HW exec time: {res.exec_time_ns} ns")
        if res.instructions_and_trace is not None:
            print("trace:", res.instructions_and_trace[1])
        _CACHE["last_result"] = res

    B = len(in_maps)
    outs = []
    for i, c in enumerate(CH):
        o = np.stack([res.results[s][f"o{i + 1}"] for s in range(B)])
        outs.append(o.reshape(B, c, 32, 32).astype(np.float32))
    return tuple(outs)
